# revision 1
# baseline (speedup 1.0000x reference)
"""Trainium2 Bass kernel for nn_CausalSelfAttention (BitLinear QKV/proj +
RMSNorm + RoPE + GQA causal attention), 8-core SPMD.

Sharding: data-parallel over batch (8 cores x 1 batch element each).
All compute on-device. Key numerics:
  - BitLinear matmuls run in bf16 on integer-valued operands -> exact.
  - Attention matmuls in float32r (~1.5e-4 component error).
  - rms_norm absorbs the activation-quant scale gamma*alpha exactly
    (per-head scale invariance), so q/k projections need no rescaling.
  - round() implemented exactly via the +/-1.5*2^23 magic constant.
"""
import numpy as np

import concourse.bass as bass
import concourse.tile as tile
import concourse.bacc as bacc
from concourse import mybir
from concourse.bass_utils import run_bass_kernel_spmd

F32 = mybir.dt.float32
F32R = mybir.dt.float32r
BF16 = mybir.dt.bfloat16
AF = mybir.ActivationFunctionType
OP = mybir.AluOpType
AX = mybir.AxisListType

DIM = 1024
NH = 16
NKV = 4
HD = 64
S = 1024
N_CORES = 8
ROPE_BASE = 10000.0
EPS8 = 1e-8
EPSF = float(np.finfo(np.float32).eps)  # 2**-23
MAGIC = float(np.float32(1.5 * 2 ** 23))
MASKVAL = -1024.0
SCALE = 1.0 / 8.0  # 1/sqrt(HD)

_CACHE = {}


def _consts():
    inv_freq = (1.0 / ROPE_BASE ** (np.arange(0, HD, 2, dtype=np.float32)
                                    / np.float32(HD))).astype(np.float32)
    freqs = np.outer(np.arange(S, dtype=np.float32), inv_freq)
    cos, sin = np.cos(freqs).astype(np.float32), np.sin(freqs).astype(np.float32)
    CT = np.zeros((128, S), dtype=np.float32)
    ST = np.zeros((128, S), dtype=np.float32)
    for p in range(128):
        i = p & 31
        CT[p] = cos[:, i]
        ST[p] = sin[:, i] * (1.0 if (p & 32) == 0 else -1.0)
    ident = np.eye(128, dtype=np.float32)
    maska = np.zeros((128, 128), dtype=np.float32)
    for r in range(128):
        maska[r, r + 1:] = MASKVAL  # out[i,j] = A[j,i] = MASK iff j<i (sq<sk)
    eblk = np.zeros((128, 2), dtype=np.float32)
    eblk[:64, 0] = 1.0
    eblk[64:, 1] = 1.0
    return CT, ST, ident, maska, eblk, eblk.T.copy()


def build(debug=False):
    nc = bacc.Bacc("TRN2", target_bir_lowering=False, debug=False,
                   num_devices=N_CORES)
    h = {
        "x": nc.dram_tensor("x", [S, DIM], F32, kind="ExternalInput"),
        "Wq": nc.dram_tensor("Wq", [DIM, DIM], F32, kind="ExternalInput"),
        "Wk": nc.dram_tensor("Wk", [NKV * HD, DIM], F32, kind="ExternalInput"),
        "Wv": nc.dram_tensor("Wv", [NKV * HD, DIM], F32, kind="ExternalInput"),
        "Wp": nc.dram_tensor("Wp", [DIM, DIM], F32, kind="ExternalInput"),
        "qg": nc.dram_tensor("q_gain", [NH], F32, kind="ExternalInput"),
        "out": nc.dram_tensor("out", [S, DIM], F32, kind="ExternalOutput"),
    }
    CTn, STn, IDn, MAn, EBn, EBTn = _consts()
    d = {
        "ct": nc.inline_tensor(CTn, name="ct_c"),
        "st": nc.inline_tensor(STn, name="st_c"),
        "idf": nc.inline_tensor(IDn, name="idf_c"),
        "ma": nc.inline_tensor(MAn, name="ma_c"),
        "eb": nc.inline_tensor(EBn, name="eb_c"),
        "ebt": nc.inline_tensor(EBTn, name="ebt_c"),
        "ones": nc.inline_tensor(np.ones((128, 128), np.float32), name="on_c"),
    }
    dbg = None
    if debug:
        dbg = {}
        for nm, shp, dt_ in [
            ("xT", [128, 8, S], BF16), ("gam", [128, 8], F32),
            ("wqT", [128, 8, DIM], BF16), ("wkT", [128, 8, 256], BF16),
            ("wvT", [128, 8, 256], BF16), ("wpT", [128, 8, DIM], BF16),
            ("vbuf", [128, 8, NKV, HD + 1], F32),
            ("qT", [8, 128, S], F32), ("kT", [2, 128, S], F32),
            ("yT", [8, 128, S], F32), ("m1", [128, S], F32),
            ("givr", [1, S], F32), ("gyap", [128, 8], F32),
            ("yint", [128, 8, S], BF16),
            ("ex0", [128, S], F32), ("scale16", [2, 8], F32),
            ("wf_raw", [128, DIM], F32), ("wf_scl", [128, DIM], F32),
            ("wf_clp", [128, DIM], F32), ("wt0", [128, DIM], BF16),
        ]:
            dbg[nm] = nc.dram_tensor("dbg_" + nm, shp, dt_,
                                     kind="ExternalOutput")
    with tile.TileContext(nc) as tc, \
            nc.allow_low_precision(reason="f32r intermediates are intended"):
        _emit(nc, tc, h, d, dbg)
    nc.finalize()
    return nc


def _emit(nc, tc, h, dd, dbg=None):
    def dump(name, ap):
        if dbg is not None and name in dbg:
            nc.sync.dma_start(dbg[name].ap(), ap)
    from contextlib import ExitStack
    ctx = ExitStack()
    cpool = ctx.enter_context(tc.tile_pool(name="consts", bufs=1))
    small = ctx.enter_context(tc.tile_pool(name="small", bufs=1))
    stage = ctx.enter_context(tc.tile_pool(name="stage", bufs=2))
    wbig = ctx.enter_context(tc.tile_pool(name="wbig", bufs=3))
    wkv = ctx.enter_context(tc.tile_pool(name="wkv", bufs=1))
    i8 = ctx.enter_context(tc.tile_pool(name="i8", bufs=1))
    big = ctx.enter_context(tc.tile_pool(name="big", bufs=13))
    vpool = ctx.enter_context(tc.tile_pool(name="vpool", bufs=1))
    ep2 = ctx.enter_context(tc.tile_pool(name="ep2", bufs=2))
    ep1 = ctx.enter_context(tc.tile_pool(name="ep1", bufs=1))
    opool = ctx.enter_context(tc.tile_pool(name="opool", bufs=2))
    ps = ctx.enter_context(tc.tile_pool(name="ps", bufs=8, space="PSUM"))

    def psum(p_, f_):
        t = ps.tile([p_, f_], F32, tag="ps", name="pst")
        return t

    # ---------------- constants ----------------
    CT = cpool.tile([128, S], F32)
    nc.sync.dma_start(CT[:], dd["ct"].ap())
    STt = cpool.tile([128, S], F32)
    nc.sync.dma_start(STt[:], dd["st"].ap())
    IDF = cpool.tile([128, 128], F32)
    nc.sync.dma_start(IDF[:], dd["idf"].ap())
    IDB = cpool.tile([128, 128], BF16)
    nc.gpsimd.dma_start(IDB[:], dd["idf"].ap())  # cast f32->bf16
    MASKA = cpool.tile([128, 128], BF16)
    nc.gpsimd.dma_start(MASKA[:], dd["ma"].ap())
    EBLK = cpool.tile([128, 2], F32R)
    nc.gpsimd.dma_start(EBLK[:], dd["eb"].ap())
    EBT = cpool.tile([2, 128], F32R)
    nc.gpsimd.dma_start(EBT[:], dd["ebt"].ap())
    ONES_K = cpool.tile([128, 1], F32)
    nc.gpsimd.dma_start(ONES_K[:], dd["ones"].ap()[:, 0:1])
    ONES_R = cpool.tile([1, 128], F32R)
    nc.gpsimd.dma_start(ONES_R[:], dd["ones"].ap()[0:1, :])
    ONES_CR = cpool.tile([128, 1], F32R)
    nc.gpsimd.dma_start(ONES_CR[:], dd["ones"].ap()[:, 0:1])
    ONES_RF = cpool.tile([1, 128], F32)
    nc.sync.dma_start(ONES_RF[:], dd["ones"].ap()[0:1, :])

    def bcast128(dst, src_11):
        # [1,1] -> [128,1] broadcast via K=1 PE matmul (ones column)
        pb = psum(128, 1)
        nc.tensor.matmul(pb[:], ONES_RF[:], src_11, start=True, stop=True)
        nc.vector.tensor_copy(dst, pb[:])

    # ---------------- q_gain -> rms scale/bias [16,1] ----------------
    qg = small.tile([2, 8], F32)  # [head-in-tile, m-tile]
    nc.sync.dma_start(qg[:], h["qg"].ap().rearrange("(a b) -> b a", b=2))
    g2 = small.tile([2, 8], F32)
    nc.vector.tensor_tensor(out=g2[:], in0=qg[:], in1=qg[:], op=OP.mult)
    c64 = small.tile([2, 8], F32)
    nc.vector.memset(c64[:], 1.0 / 64.0)
    ceps = small.tile([2, 8], F32)
    nc.vector.memset(ceps[:], EPSF)
    rg2 = small.tile([2, 8], F32)
    nc.vector.reciprocal(out=rg2[:], in_=g2[:])
    scale16 = small.tile([2, 8], F32)
    nc.vector.tensor_scalar(out=scale16[:], in0=rg2[:], scalar1=1.0 / 64.0,
                            scalar2=None, op0=OP.mult)
    bias16 = small.tile([2, 8], F32)
    nc.vector.tensor_scalar(out=bias16[:], in0=rg2[:], scalar1=EPSF,
                            scalar2=None, op0=OP.mult)

    # ---------------- x: per-token int8 fake-quant + transpose --------
    xT = i8.tile([128, 8, S], BF16, tag="i8")  # [d-part, d-tile, s]
    gam = small.tile([128, 8], F32)
    for st in range(8):
        xf = stage.tile([128, DIM], F32, tag="wf32")
        nc.sync.dma_start(xf[:], h["x"].ap()[st * 128:(st + 1) * 128, :])
        gmax = small.tile([128, 1], F32, tag="gmax")
        nc.vector.tensor_reduce(out=gmax[:], in_=xf[:], axis=AX.X,
                                op=OP.max, apply_absolute_value=True)
        nc.vector.tensor_scalar(out=gam[:, st:st + 1], in0=gmax[:],
                                scalar1=EPS8, scalar2=float(np.float32(1.0) / np.float32(127.0)),
                                op0=OP.max, op1=OP.mult)
        ginv = small.tile([128, 1], F32, tag="ginv")
        nc.vector.reciprocal(out=ginv[:], in_=gam[:, st:st + 1])
        nc.scalar.activation(out=xf[:], in_=xf[:], func=AF.Copy,
                             bias=MAGIC, scale=ginv[:])
        xi = stage.tile([128, DIM], BF16, tag="wt")
        nc.gpsimd.tensor_scalar(out=xi[:], in0=xf[:], scalar1=MAGIC,
                                scalar2=None, op0=OP.subtract)
        for d in range(8):
            ptx = ps.tile([128, 128], BF16, tag="ps", name="ptx")
            nc.tensor.transpose(ptx[:], xi[:, d * 128:(d + 1) * 128], IDB[:])
            nc.vector.tensor_copy(xT[:, d, st * 128:(st + 1) * 128], ptx[:])

    dump("xT", xT[:])
    dump("gam", gam[:])
    # ---------------- weight prep ----------------
    def prep_w(w_h, R, wT_tile, need_alpha=False, perm_tile=None):
        name = w_h.name
        asum = small.tile([128, R], F32, tag=f"as{name}")
        for r in range(R):
            wf = stage.tile([128, DIM], F32, tag="wf32")
            nc.sync.dma_start(wf[:], w_h.ap()[r * 128:(r + 1) * 128, :])
            nc.scalar.activation(out=wf[:], in_=wf[:], func=AF.Abs,
                                 accum_out=asum[:, r:r + 1])
        atot = small.tile([128, 1], F32, tag=f"at{name}")
        if R > 1:
            nc.vector.tensor_reduce(out=atot[:], in_=asum[:], axis=AX.X,
                                    op=OP.add)
        else:
            nc.vector.tensor_copy(atot[:], asum[:])
        pa = psum(1, 1)
        nc.tensor.matmul(pa[:], ONES_K[:], atot[:], start=True, stop=True)
        alpha = small.tile([1, 1], F32, tag=f"al{name}")
        nc.vector.tensor_scalar(out=alpha[:], in0=pa[:],
                                scalar1=1.0 / (R * 128 * DIM), scalar2=EPS8,
                                op0=OP.mult, op1=OP.max)
        ainv = small.tile([1, 1], F32, tag=f"ai{name}")
        nc.vector.reciprocal(out=ainv[:], in_=alpha[:])
        ainv_bc = small.tile([128, 1], F32, tag=f"aib{name}")
        bcast128(ainv_bc[:], ainv[0:1, 0:1])
        alpha_bc = None
        if need_alpha:
            alpha_bc = small.tile([128, 1], F32, tag=f"ab{name}")
            bcast128(alpha_bc[:], alpha[0:1, 0:1])
        for r in range(R):
            wf = stage.tile([128, DIM], F32, tag="wf32")
            nc.sync.dma_start(wf[:], w_h.ap()[r * 128:(r + 1) * 128, :])
            if name == "Wp" and r == 0:
                dump("wf_raw", wf[:])
            nc.scalar.activation(out=wf[:], in_=wf[:], func=AF.Copy,
                                 bias=0.0, scale=ainv_bc[:])
            if name == "Wp" and r == 0:
                dump("wf_scl", wf[:])
            nc.vector.tensor_scalar(out=wf[:], in0=wf[:], scalar1=1.0,
                                    scalar2=-1.0, op0=OP.min, op1=OP.max)
            if name == "Wp" and r == 0:
                dump("wf_clp", wf[:])
            wt = stage.tile([128, DIM], BF16, tag="wt")
            nc.vector.tensor_scalar(out=wt[:], in0=wf[:], scalar1=MAGIC,
                                    scalar2=MAGIC, op0=OP.add,
                                    op1=OP.subtract)
            if name == "Wp" and r == 0:
                dump("wt0", wt[:])
            for d in range(8):
                ptw = ps.tile([128, 128], BF16, tag="ps", name="ptw")
                nc.tensor.transpose(ptw[:], wt[:, d * 128:(d + 1) * 128],
                                    IDB[:])
                nc.vector.tensor_copy(wT_tile[:, d, r * 128:(r + 1) * 128],
                                      ptw[:])
        if perm_tile is not None:
            src = wT_tile[:].rearrange("p d (b h c) -> p d b h c", h=2, c=32)
            dst = perm_tile[:].rearrange("p d (b h c) -> p d b h c", h=2, c=32)
            nc.gpsimd.tensor_copy(dst[:, :, :, 1, :], src[:, :, :, 0, :])
            nc.gpsimd.tensor_copy(dst[:, :, :, 0, :], src[:, :, :, 1, :])
        return alpha_bc

    wqT = wbig.tile([128, 8, DIM], BF16, tag="wbig")
    wqTs = wbig.tile([128, 8, DIM], BF16, tag="wbig")
    wkT = wkv.tile([128, 8, NKV * HD], BF16, tag="wkT")
    wkTs = wkv.tile([128, 8, NKV * HD], BF16, tag="wkTs")
    wvT = wkv.tile([128, 8, NKV * HD], BF16, tag="wvT")
    prep_w(h["Wq"], 8, wqT, perm_tile=wqTs)
    prep_w(h["Wk"], 2, wkT, perm_tile=wkTs)
    av_bc = prep_w(h["Wv"], 2, wvT, need_alpha=True)

    dump("wqT", wqT[:])
    dump("wkT", wkT[:])
    dump("wvT", wvT[:])
    # ---------------- V projection -> vbuf [s, kvh, 64+1] f32r --------
    vbuf = vpool.tile([128, 8, NKV, HD + 1], F32R, tag="vbuf")
    for st in range(8):
        pv = psum(128, NKV * HD)
        for d in range(8):
            nc.tensor.matmul(pv[:], xT[:, d, st * 128:(st + 1) * 128],
                             wvT[:, d, :], start=(d == 0), stop=(d == 7))
        vg = small.tile([128, 1], F32, tag="vg")
        nc.vector.tensor_tensor(out=vg[:], in0=gam[:, st:st + 1],
                                in1=av_bc[:], op=OP.mult)
        nc.vector.tensor_scalar(
            out=vbuf[:, st, :, 0:HD],
            in0=pv[:].rearrange("p (h d) -> p h d", d=HD),
            scalar1=vg[:], scalar2=None, op0=OP.mult)
        nc.vector.tensor_copy(vbuf[:, st, :, HD:HD + 1],
                              ONES_CR[:].broadcast_to((128, 4, 1)))

    # ---------------- Q/K proj + rms-norm + rope -> qT/kT f32r --------
    def qk_post(psA, psB, dest, sl, s16, b16):
        sq = ep2.tile([128, 512], F32R, tag="t1")
        nc.scalar.activation(out=sq[:], in_=psA[:], func=AF.Square)
        ps2 = psum(2, 512)
        nc.tensor.matmul(ps2[:], EBLK[:], sq[:], start=True, stop=True)
        sr = ep2.tile([2, 512], F32, tag="sr")
        nc.scalar.activation(out=sr[:], in_=ps2[:], func=AF.Sqrt,
                             bias=b16, scale=s16)
        rs = ep2.tile([2, 512], F32R, tag="rs")
        nc.vector.reciprocal(out=rs[:], in_=sr[:])
        prs = psum(128, 512)
        nc.tensor.matmul(prs[:], EBT[:], rs[:], start=True, stop=True)
        t1 = ep2.tile([128, 512], F32, tag="t1")
        nc.vector.tensor_tensor(out=t1[:], in0=psA[:], in1=CT[:, sl],
                                op=OP.mult)
        t2 = ep2.tile([128, 512], F32, tag="t2")
        nc.vector.tensor_tensor(out=t2[:], in0=psB[:], in1=STt[:, sl],
                                op=OP.mult)
        nc.vector.tensor_tensor(out=t1[:], in0=t1[:], in1=t2[:], op=OP.add)
        nc.vector.tensor_tensor(out=dest[:, sl], in0=t1[:], in1=prs[:],
                                op=OP.mult)

    qT = [big.tile([128, S], F32R, tag="b4k", name=f"qT{i}") for i in range(8)]
    kT = [big.tile([128, S], F32R, tag="b4k", name=f"kT{i}") for i in range(2)]

    def qk_proj(wT, wTs, dest_list, nmt, s16_fn, b16_fn):
        for mt in range(nmt):
            for half in range(2):
                sl = slice(half * 512, half * 512 + 512)
                psA = psum(128, 512)
                psB = psum(128, 512)
                for d in range(8):
                    nc.tensor.matmul(psA[:],
                                     wT[:, d, mt * 128:(mt + 1) * 128],
                                     xT[:, d, sl],
                                     start=(d == 0), stop=(d == 7))
                for d in range(8):
                    nc.tensor.matmul(psB[:],
                                     wTs[:, d, mt * 128:(mt + 1) * 128],
                                     xT[:, d, sl],
                                     start=(d == 0), stop=(d == 7))
                qk_post(psA, psB, dest_list[mt], sl, s16_fn(mt), b16_fn(mt))

    qk_proj(wqT, wqTs, qT, 8,
            lambda mt: scale16[0:2, mt:mt + 1],
            lambda mt: bias16[0:2, mt:mt + 1])
    qk_proj(wkT, wkTs, kT, 2, lambda mt: 1.0 / 64.0,
            lambda mt: ceps[0:2, 0:1])

    dump("vbuf", vbuf[:].bitcast(F32))
    if dbg is not None:
        for _i in range(8):
            nc.sync.dma_start(dbg["qT"].ap()[_i], qT[_i][:].bitcast(F32))
        for _i in range(2):
            nc.sync.dma_start(dbg["kT"].ap()[_i], kT[_i][:].bitcast(F32))
        dump("scale16", scale16[:])
    # partition-swapped copy of kT so every head's k slice can match the
    # base partition of its q slice (PE requires equal base partitions)
    kTb = [big.tile([128, S], F32R, tag="b4k", name=f"kTb{i}")
           for i in range(2)]
    for mt in range(2):
        nc.sync.dma_start(kTb[mt][0:64, :], kT[mt][64:128, :])
        nc.sync.dma_start(kTb[mt][64:128, :], kT[mt][0:64, :])

    # ---------------- Wp prep (wbig slots freed by now) ---------------
    wpT = wbig.tile([128, 8, DIM], BF16, tag="wbig")
    ap_bc = prep_w(h["Wp"], 8, wpT, need_alpha=True)

    # ---------------- attention ----------------
    yT = []
    for hh in range(NH):
        if hh % 2 == 0:
            yT.append(big.tile([128, S], F32, tag="b4k", name=f"yT{hh//2}"))
        ytile, ypo = yT[hh // 2], 64 * (hh % 2)
        hk = hh // 4
        qtile, qpo = qT[hh // 2], 64 * (hh % 2)
        if qpo == 64 * (hk % 2):
            ktile, kpo = kT[hk // 2], qpo
        else:
            ktile, kpo = kTb[hk // 2], qpo

        pyh = [psum(HD + 1, 512), psum(HD + 1, 512)]
        for i in range(8):
            ex = ep2.tile([128, S], F32R, tag="expT")
            for half in range(2):
                lo = max(half * 512, i * 128)
                hi = half * 512 + 512
                if lo >= hi:
                    continue
                psc = psum(128, 512)
                diag = (i * 128 == lo) and (i >= half * 4) and (i < half * 4 + 4)
                nc.tensor.matmul(psc[:, lo - half * 512:512],
                                 ktile[kpo:kpo + 64, i * 128:(i + 1) * 128],
                                 qtile[qpo:qpo + 64, lo:hi],
                                 start=True, stop=not diag)
                if diag:
                    nc.tensor.matmul(
                        psc[:, lo - half * 512:lo - half * 512 + 128],
                        MASKA[:], IDB[:], start=False, stop=True)
                nc.scalar.activation(out=ex[:, lo:hi],
                                     in_=psc[:, lo - half * 512:512],
                                     func=AF.Exp, scale=SCALE)
                nc.tensor.matmul(pyh[half][:, lo - half * 512:512],
                                 vbuf[:, i, hk, :], ex[:, lo:hi],
                                 start=(i == 0),
                                 stop=(i == 7 or (half == 0 and i == 3)))
            if hh == 0 and i == 0 and dbg is not None:
                nc.sync.dma_start(dbg["ex0"].ap(), ex[:].bitcast(F32))
        for half in range(2):
            sl = slice(half * 512, half * 512 + 512)
            dr = ep2.tile([1, 512], F32R, tag="dr")
            nc.vector.reciprocal(out=dr[:], in_=pyh[half][HD:HD + 1, :])
            pdb = psum(HD, 512)
            nc.tensor.matmul(pdb[:], ONES_R[0:1, 0:HD], dr[:],
                             start=True, stop=True)
            nc.scalar.activation(out=ytile[ypo:ypo + HD, sl],
                                 in_=pyh[half][0:HD, :], func=AF.Copy)
            nc.vector.tensor_tensor(out=ytile[ypo:ypo + HD, sl],
                                    in0=ytile[ypo:ypo + HD, sl],
                                    in1=pdb[:], op=OP.mult)

    if dbg is not None:
        for _i in range(8):
            nc.sync.dma_start(dbg["yT"].ap()[_i], yT[_i][:])
    # ---------------- y fake-quant (gamma over d, per token) ----------
    m1 = ep1.tile([128, S], F32, tag="m1")
    nc.scalar.activation(out=m1[:], in_=yT[0][:], func=AF.Abs)
    for d in range(1, 8):
        ab = ep2.tile([128, S], F32, tag="expT")
        nc.scalar.activation(out=ab[:], in_=yT[d][:], func=AF.Abs)
        nc.vector.tensor_tensor(out=m1[:], in0=m1[:], in1=ab[:], op=OP.max)
    gyap = small.tile([128, 8], F32)
    givr = ep1.tile([1, S], F32R, tag="givr")
    for c in range(8):
        ptp = psum(128, 128)
        nc.tensor.transpose(ptp[:], m1[:, c * 128:(c + 1) * 128], IDF[:])
        my = small.tile([128, 1], F32, tag="my")
        nc.vector.tensor_reduce(out=my[:], in_=ptp[:], axis=AX.X, op=OP.max)
        gmy = small.tile([128, 1], F32, tag="gmy")
        nc.vector.tensor_scalar(out=gmy[:], in0=my[:], scalar1=EPS8,
                                scalar2=float(np.float32(1.0) / np.float32(127.0)),
                                op0=OP.max, op1=OP.mult)
        giv = small.tile([128, 1], F32, tag="giv")
        nc.vector.reciprocal(out=giv[:], in_=gmy[:])
        nc.vector.tensor_tensor(out=gyap[:, c:c + 1], in0=gmy[:],
                                in1=ap_bc[:], op=OP.mult)
        pgi = psum(1, 128)
        nc.tensor.transpose(pgi[:], giv[:], IDF[:])
        nc.vector.tensor_copy(givr[:, c * 128:(c + 1) * 128], pgi[:])

    dump("m1", m1[:])
    dump("givr", givr[:].bitcast(F32))
    dump("gyap", gyap[:])
    yint = i8.tile([128, 8, S], BF16, tag="i8")  # reuses xT slot
    for half in range(2):
        sl = slice(half * 512, half * 512 + 512)
        pg = psum(128, 512)
        nc.tensor.matmul(pg[:], ONES_R[:], givr[:, sl], start=True, stop=True)
        for d in range(8):
            z = ep2.tile([128, 512], F32, tag="zq")
            nc.vector.tensor_tensor(out=z[:], in0=yT[d][:, sl], in1=pg[:],
                                    op=OP.mult)
            nc.gpsimd.tensor_scalar(out=yint[:, d, sl], in0=z[:],
                                    scalar1=MAGIC, scalar2=MAGIC,
                                    op0=OP.add, op1=OP.subtract)

    dump("yint", yint[:])
    dump("wpT", wpT[:])
    # ---------------- output projection ----------------
    for st in range(8):
        for half in range(2):
            sl = slice(half * 512, half * 512 + 512)
            po = psum(128, 512)
            for d in range(8):
                nc.tensor.matmul(po[:], yint[:, d, st * 128:(st + 1) * 128],
                                 wpT[:, d, sl], start=(d == 0), stop=(d == 7))
            ot = opool.tile([128, 512], F32, tag="ot")
            nc.vector.tensor_scalar(out=ot[:], in0=po[:],
                                    scalar1=gyap[:, st:st + 1], scalar2=None,
                                    op0=OP.mult)
            nc.sync.dma_start(h["out"].ap()[st * 128:(st + 1) * 128, sl],
                              ot[:])
    ctx.close()


def kernel(**inputs):
    if "nc" not in _CACHE:
        _CACHE["nc"] = build()
    nc = _CACHE["nc"]
    x = np.ascontiguousarray(inputs["x"], dtype=np.float32)
    shared = {k: np.ascontiguousarray(inputs[k], dtype=np.float32)
              for k in ("Wq", "Wk", "Wv", "Wp")}
    qgain = np.ascontiguousarray(inputs["q_gain"], dtype=np.float32)
    in_maps = [dict(x=np.ascontiguousarray(x[b]), q_gain=qgain, **shared)
               for b in range(N_CORES)]
    res = run_bass_kernel_spmd(nc, in_maps, core_ids=list(range(N_CORES)))
    out = np.stack([res.results[b]["out"] for b in range(N_CORES)], axis=0)
    return out.astype(np.float32)



# revision 12
# speedup vs baseline: 1.8690x; 1.8690x over previous
"""Trainium2 Bass kernel for nn_CausalSelfAttention (BitLinear QKV/proj +
RMSNorm + RoPE + GQA causal attention), 8-core SPMD.

Sharding: data-parallel over batch (8 cores x 1 batch element each).

v2 restructure vs baseline:
  - RoPE partner values via a single 128x128 permutation matmul (PSWAP)
    on the projected q/k instead of a second full K=1024 projection.
  - All rms-norm square-sums collected into one PSUM tile (QS2) via
    zero-padded selector matmuls; ONE batched sqrt + ONE batched
    reciprocal (vector reciprocal cost scales with free size only).
  - Softmax denominators gathered into Dall; ONE batched reciprocal;
    inverse broadcast to head partitions via selector matmuls on PE.
  - No gpsimd bulk ops (round-magic on vector 2-op tensor_scalar).
  - PE transposes packed 8-per-PSUM-bank, single vector copy out.
  - exp output in bf16 (bf16 moving operand for the pyh matmuls).
  - Scoped PSUM/SBUF pools so phases reuse space.
"""
import numpy as np

import concourse.bass as bass
import concourse.tile as tile
import concourse.bacc as bacc
from concourse import mybir
from concourse.bass_utils import run_bass_kernel_spmd

F32 = mybir.dt.float32
F32R = mybir.dt.float32r
BF16 = mybir.dt.bfloat16
AF = mybir.ActivationFunctionType
OP = mybir.AluOpType
AX = mybir.AxisListType

DIM = 1024
NH = 16
NKV = 4
HD = 64
S = 1024
N_CORES = 8
ROPE_BASE = 10000.0
EPS8 = 1e-8
EPSF = float(np.finfo(np.float32).eps)  # 2**-23
MAGIC = float(np.float32(1.5 * 2 ** 23))
MASKVAL = -1024.0
SCALE = 1.0 / 8.0  # 1/sqrt(HD)
INV127 = float(np.float32(1.0) / np.float32(127.0))

_CACHE = {}


def _consts():
    inv_freq = (1.0 / ROPE_BASE ** (np.arange(0, HD, 2, dtype=np.float32)
                                    / np.float32(HD))).astype(np.float32)
    freqs = np.outer(np.arange(S, dtype=np.float32), inv_freq)
    cos, sin = np.cos(freqs).astype(np.float32), np.sin(freqs).astype(np.float32)
    CT = np.zeros((128, S), dtype=np.float32)
    ST = np.zeros((128, S), dtype=np.float32)
    for p in range(128):
        i = p & 31
        CT[p] = cos[:, i]
        ST[p] = sin[:, i] * (1.0 if (p & 32) == 0 else -1.0)
    ident = np.eye(128, dtype=np.float32)
    maska = np.zeros((128, 128), dtype=np.float32)
    for r in range(128):
        maska[r, r + 1:] = MASKVAL  # out[i,j] = A[j,i] = MASK iff j<i (sq<sk)
    # partition-swap permutation: (PSWAP^T q)[m] = q[m ^ 32]
    pswap = np.zeros((128, 128), dtype=np.float32)
    for m in range(128):
        pswap[m ^ 32, m] = 1.0
    # EBPS[p, mt, j] = 1 iff j == 2*mt + (p>=64); lhsT for QS2 accumulation
    ebps = np.zeros((128, 10, 20), dtype=np.float32)
    for p in range(128):
        for mt in range(10):
            ebps[p, mt, 2 * mt + (1 if p >= 64 else 0)] = 1.0
    # SEL20[r, mt, p] = 1 iff r == 2*mt + (p>=64); lhsT to broadcast row
    # (2*mt + p-half) of a [20, n] tile onto 128 partitions
    sel20 = np.zeros((20, 10, 128), dtype=np.float32)
    for mt in range(10):
        for p in range(128):
            sel20[2 * mt + (1 if p >= 64 else 0), mt, p] = 1.0
    return CT, ST, ident, maska, pswap, ebps, sel20


def build():
    nc = bacc.Bacc("TRN2", target_bir_lowering=False, debug=False,
                   num_devices=N_CORES)
    h = {
        "x": nc.dram_tensor("x", [S, DIM], F32, kind="ExternalInput"),
        "Wq": nc.dram_tensor("Wq", [DIM, DIM], F32, kind="ExternalInput"),
        "Wk": nc.dram_tensor("Wk", [NKV * HD, DIM], F32, kind="ExternalInput"),
        "Wv": nc.dram_tensor("Wv", [NKV * HD, DIM], F32, kind="ExternalInput"),
        "Wp": nc.dram_tensor("Wp", [DIM, DIM], F32, kind="ExternalInput"),
        "qg": nc.dram_tensor("q_gain", [NH], F32, kind="ExternalInput"),
        "out": nc.dram_tensor("out", [S, DIM], F32, kind="ExternalOutput"),
    }
    CTn, STn, IDn, MAn, PSn, EBPSn, SEL20n = _consts()
    d = {
        "ct": nc.inline_tensor(CTn, name="ct_c"),
        "st": nc.inline_tensor(STn, name="st_c"),
        "idf": nc.inline_tensor(IDn, name="idf_c"),
        "ma": nc.inline_tensor(MAn, name="ma_c"),
        "pswap": nc.inline_tensor(PSn, name="pswap_c"),
        "ebps": nc.inline_tensor(EBPSn.reshape(128, 200), name="ebps_c"),
        "sel20": nc.inline_tensor(SEL20n.reshape(20, 1280), name="sel20_c"),
        "ones": nc.inline_tensor(np.ones((128, 128), np.float32), name="on_c"),
    }
    with tile.TileContext(nc) as tc, \
            nc.allow_low_precision(reason="f32r intermediates are intended"):
        _emit(nc, tc, h, d)
    nc.finalize()
    return nc


def _emit(nc, tc, h, dd):
    from contextlib import ExitStack
    ctx = ExitStack()
    cpool = ctx.enter_context(tc.tile_pool(name="consts", bufs=1))
    small = ctx.enter_context(tc.tile_pool(name="small", bufs=1))
    wbig = ctx.enter_context(tc.tile_pool(name="wbig", bufs=1))
    wkv = ctx.enter_context(tc.tile_pool(name="wkv", bufs=1))
    i8 = ctx.enter_context(tc.tile_pool(name="i8", bufs=1))
    big = ctx.enter_context(tc.tile_pool(name="big", bufs=1))
    vpool = ctx.enter_context(tc.tile_pool(name="vpool", bufs=1))
    ep = ctx.enter_context(tc.tile_pool(name="ep", bufs=2))
    expool = ctx.enter_context(tc.tile_pool(name="expool", bufs=2))
    opool = ctx.enter_context(tc.tile_pool(name="opool", bufs=2))
    # scoped pools (closed mid-kernel so later phases reuse the space)
    stage_ctx = ExitStack()
    stage = stage_ctx.enter_context(tc.tile_pool(name="stage", bufs=1))
    ps12_ctx = ExitStack()
    ps12 = ps12_ctx.enter_context(tc.tile_pool(name="ps12", bufs=1,
                                               space="PSUM"))

    # ---------------- constants ----------------
    CT = cpool.tile([128, S], F32)
    nc.sync.dma_start(CT[:], dd["ct"].ap())
    STt = cpool.tile([128, S], F32)
    nc.sync.dma_start(STt[:], dd["st"].ap())
    IDF = cpool.tile([128, 128], F32)
    nc.sync.dma_start(IDF[:], dd["idf"].ap())
    IDB = cpool.tile([128, 128], BF16)
    nc.gpsimd.dma_start(IDB[:], dd["idf"].ap())  # cast f32->bf16
    MASKA = cpool.tile([128, 128], BF16)
    nc.gpsimd.dma_start(MASKA[:], dd["ma"].ap())
    PSWAP = cpool.tile([128, 128], F32R)
    nc.gpsimd.dma_start(PSWAP[:], dd["pswap"].ap())
    EBPS = cpool.tile([128, 10, 20], F32R)
    nc.gpsimd.dma_start(EBPS[:], dd["ebps"].ap().rearrange(
        "p (m j) -> p m j", j=20))
    SEL20 = cpool.tile([20, 10, 128], F32R)
    nc.gpsimd.dma_start(SEL20[:], dd["sel20"].ap().rearrange(
        "r (m p) -> r m p", p=128))
    ONES_K = cpool.tile([128, 1], F32)
    nc.gpsimd.dma_start(ONES_K[:], dd["ones"].ap()[:, 0:1])
    ONES_CR = cpool.tile([128, 1], F32R)
    nc.gpsimd.dma_start(ONES_CR[:], dd["ones"].ap()[:, 0:1])
    ONES_RF = cpool.tile([1, 128], F32)
    nc.sync.dma_start(ONES_RF[:], dd["ones"].ap()[0:1, :])
    ONES8 = cpool.tile([8, 128], F32R)
    nc.gpsimd.dma_start(ONES8[:], dd["ones"].ap()[0:8, :])
    IDR = IDF[:].bitcast(F32R)

    def pa(shape, dtype=F32, name="pa_t"):
        return ps12.tile(shape, dtype, tag="pa", name=name, bufs=4)

    def bcast128(dst, src_11):
        # [1,1] -> [128,1] broadcast via K=1 PE matmul (ones row)
        pb = pa([128, 1], name="pb_t")
        nc.tensor.matmul(pb[:], ONES_RF[:], src_11, start=True, stop=True)
        nc.vector.tensor_copy(dst, pb[:])

    # ---------------- q_gain -> scale20/bias20 ----------------
    qgt = small.tile([16, 1], F32)
    nc.sync.dma_start(qgt[:], h["qg"].ap().rearrange("(a b) -> a b", b=1))
    qg2 = small.tile([16, 1], F32)
    nc.vector.tensor_tensor(out=qg2[:], in0=qgt[:], in1=qgt[:], op=OP.mult)
    rg20 = small.tile([20, 1], F32)
    nc.vector.tensor_copy(rg20[:], ONES_K[0:20, :])
    nc.vector.reciprocal(out=rg20[0:16], in_=qg2[:])
    scale20 = small.tile([20, 1], F32)
    nc.vector.tensor_scalar(out=scale20[:], in0=rg20[:], scalar1=1.0 / 64.0,
                            scalar2=None, op0=OP.mult)
    bias20 = small.tile([20, 1], F32)
    nc.vector.tensor_scalar(out=bias20[:], in0=rg20[:], scalar1=EPSF,
                            scalar2=None, op0=OP.mult)

    # ---------------- x: per-token int8 fake-quant + transpose --------
    xT = i8.tile([128, 8, S], BF16, tag="i8")  # [d-part, d-tile, s]
    gam = small.tile([128, 8], F32)
    for st in range(8):
        xf = stage.tile([128, DIM], F32, tag="xf32", bufs=2)
        nc.sync.dma_start(xf[:], h["x"].ap()[st * 128:(st + 1) * 128, :])
        gmax = small.tile([128, 1], F32, tag="gmax", bufs=2)
        nc.vector.tensor_reduce(out=gmax[:], in_=xf[:], axis=AX.X,
                                op=OP.max, apply_absolute_value=True)
        nc.vector.tensor_scalar(out=gam[:, st:st + 1], in0=gmax[:],
                                scalar1=EPS8, scalar2=INV127,
                                op0=OP.max, op1=OP.mult)
        ginv = small.tile([128, 1], F32, tag="ginv", bufs=2)
        nc.vector.reciprocal(out=ginv[:], in_=gam[:, st:st + 1])
        nc.scalar.activation(out=xf[:], in_=xf[:], func=AF.Copy,
                             bias=MAGIC, scale=ginv[:])
        xi = stage.tile([128, DIM], BF16, tag="xi", bufs=2)
        nc.vector.tensor_scalar(out=xi[:], in0=xf[:], scalar1=MAGIC,
                                scalar2=None, op0=OP.subtract)
        pt = pa([128, 8, 128], BF16, name="ptx")
        for dt_ in range(8):
            nc.tensor.transpose(pt[:, dt_, :],
                                xi[:, dt_ * 128:(dt_ + 1) * 128], IDB[:])
        nc.vector.tensor_copy(xT[:, :, st * 128:(st + 1) * 128], pt[:])

    # ---------------- weight prep ----------------
    def prep_w(w_h, R, wT_tile, need_alpha=False):
        name = w_h.name
        resident = R <= 2
        wraw = []
        asum = small.tile([128, R], F32, tag=f"as{name}", name=f"as{name}")
        for r in range(R):
            if resident:
                wf = stage.tile([128, DIM], F32, tag=f"wraw{r}", bufs=1,
                                name=f"wr{name}{r}")
                wraw.append(wf)
            else:
                wf = stage.tile([128, DIM], F32, tag="xf32", bufs=2,
                                name="wf1_t")
            nc.sync.dma_start(wf[:], w_h.ap()[r * 128:(r + 1) * 128, :])
            if resident:
                wa = stage.tile([128, DIM], F32, tag="wabs", bufs=2,
                                name="wa_t")
                nc.scalar.activation(out=wa[:], in_=wf[:], func=AF.Abs,
                                     accum_out=asum[:, r:r + 1])
            else:
                nc.scalar.activation(out=wf[:], in_=wf[:], func=AF.Abs,
                                     accum_out=asum[:, r:r + 1])
        atot = small.tile([128, 1], F32, tag=f"at{name}", name=f"at{name}")
        if R > 1:
            nc.vector.tensor_reduce(out=atot[:], in_=asum[:], axis=AX.X,
                                    op=OP.add)
        else:
            nc.vector.tensor_copy(atot[:], asum[:])
        pa_ = pa([1, 1], name="paw")
        nc.tensor.matmul(pa_[:], ONES_K[:], atot[:], start=True, stop=True)
        alpha = small.tile([1, 1], F32, tag=f"al{name}", name=f"al{name}")
        nc.vector.tensor_scalar(out=alpha[:], in0=pa_[:],
                                scalar1=1.0 / (R * 128 * DIM), scalar2=EPS8,
                                op0=OP.mult, op1=OP.max)
        ainv = small.tile([1, 1], F32, tag=f"ai{name}", name=f"ai{name}")
        nc.vector.reciprocal(out=ainv[:], in_=alpha[:])
        ainv_bc = small.tile([128, 1], F32, tag=f"aib{name}", name=f"aib{name}")
        bcast128(ainv_bc[:], ainv[0:1, 0:1])
        alpha_bc = None
        if need_alpha:
            alpha_bc = small.tile([128, 1], F32, tag=f"ab{name}",
                                  name=f"ab{name}")
            bcast128(alpha_bc[:], alpha[0:1, 0:1])
        for r in range(R):
            if resident:
                wsrc = wraw[r]
            else:
                wsrc = stage.tile([128, DIM], F32, tag="xf32", bufs=2,
                                  name="wf2_t")
                nc.sync.dma_start(wsrc[:], w_h.ap()[r * 128:(r + 1) * 128, :])
            wscl = stage.tile([128, DIM], F32, tag="wscl", bufs=2,
                              name="wscl_t")
            nc.scalar.activation(out=wscl[:], in_=wsrc[:], func=AF.Copy,
                                 bias=0.0, scale=ainv_bc[:])
            nc.vector.tensor_scalar(out=wscl[:], in0=wscl[:], scalar1=1.0,
                                    scalar2=-1.0, op0=OP.min, op1=OP.max)
            wt = stage.tile([128, DIM], BF16, tag="wt", bufs=2, name="wt_t")
            nc.vector.tensor_scalar(out=wt[:], in0=wscl[:], scalar1=MAGIC,
                                    scalar2=MAGIC, op0=OP.add,
                                    op1=OP.subtract)
            pt = pa([128, 8, 128], BF16, name="ptw")
            for dt_ in range(8):
                nc.tensor.transpose(pt[:, dt_, :],
                                    wt[:, dt_ * 128:(dt_ + 1) * 128], IDB[:])
            nc.vector.tensor_copy(wT_tile[:, :, r * 128:(r + 1) * 128], pt[:])
        return alpha_bc

    wqT = wbig.tile([128, 8, DIM], BF16, tag="w8", name="wqT")
    wkT = wkv.tile([128, 8, NKV * HD], BF16, tag="wkT")
    wvT = wkv.tile([128, 8, NKV * HD], BF16, tag="wvT")
    prep_w(h["Wq"], 8, wqT)
    prep_w(h["Wk"], 2, wkT)
    av_bc = prep_w(h["Wv"], 2, wvT, need_alpha=True)

    # ---------------- V projection -> vbuf [s, kvh, 64+1] f32r --------
    vbuf = vpool.tile([128, 8, NKV, HD + 1], F32R, tag="vbuf")
    for st in range(8):
        pv = pa([128, NKV * HD], name="pv_t")
        for d in range(8):
            nc.tensor.matmul(pv[:], xT[:, d, st * 128:(st + 1) * 128],
                             wvT[:, d, :], start=(d == 0), stop=(d == 7))
        vg = small.tile([128, 1], F32, tag="vg", bufs=2)
        nc.vector.tensor_tensor(out=vg[:], in0=gam[:, st:st + 1],
                                in1=av_bc[:], op=OP.mult)
        nc.vector.tensor_scalar(
            out=vbuf[:, st, :, 0:HD],
            in0=pv[:].rearrange("p (h d) -> p h d", d=HD),
            scalar1=vg[:], scalar2=None, op0=OP.mult)
        nc.vector.tensor_copy(vbuf[:, st, :, HD:HD + 1],
                              ONES_CR[:].broadcast_to((128, 4, 1)))

    # ---------------- Q/K proj loop 1: raw proj + square-sums ---------
    # qT[:, 0:8, :] = q rows, qT[:, 8:10, :] = k rows (pre-rope raw, then
    # overwritten in-place by loop 2 with the final rotated+normed rows)
    qT = big.tile([128, 10, S], F32R, tag="qT")
    QS2 = ps12.tile([20, S], F32, tag="q2", name="qs2_t")
    for mt in range(10):
        wTt = wqT if mt < 8 else wkT
        mtt = mt if mt < 8 else (mt - 8)
        for half in range(2):
            sl = slice(half * 512, half * 512 + 512)
            psA = pa([128, 512], name="psA_t")
            for d in range(8):
                nc.tensor.matmul(psA[:],
                                 wTt[:, d, mtt * 128:(mtt + 1) * 128],
                                 xT[:, d, sl],
                                 start=(d == 0), stop=(d == 7))
            nc.scalar.activation(out=qT[:, mt, sl], in_=psA[:], func=AF.Copy)
            sq = ep.tile([128, 512], F32R, tag="sq")
            nc.scalar.activation(out=sq[:], in_=psA[:], func=AF.Square)
            nc.tensor.matmul(QS2[:, sl], EBPS[:, mt, :], sq[:],
                             start=(mt == 0), stop=(mt == 9),
                             skip_group_check=True)
    # batched rsqrt: rs20 = 1/sqrt(QS2*scale20 + bias20)
    sr20 = small.tile([20, S], F32, tag="sr20")
    nc.scalar.activation(out=sr20[:], in_=QS2[:], func=AF.Sqrt,
                         bias=bias20[:], scale=scale20[:])
    rs20t = small.tile([20, S], F32R, tag="rs20")
    nc.vector.reciprocal(out=rs20t[:], in_=sr20[:])
    rs20 = rs20t[:]

    # ---------------- Q/K proj loop 2: rope + norm scale --------------
    for mt in range(10):
        for half in range(2):
            sl = slice(half * 512, half * 512 + 512)
            psB = pa([128, 512], name="psB_t")
            nc.tensor.matmul(psB[:], PSWAP[:], qT[:, mt, sl],
                             start=True, stop=True)
            prs = ps12.tile([128, 512], F32, tag="pr", name="prs_t", bufs=2)
            nc.tensor.matmul(prs[:], SEL20[:, mt, :], rs20[:, sl],
                             start=True, stop=True)
            t1 = ep.tile([128, 512], F32, tag="t1")
            nc.vector.tensor_tensor(out=t1[:], in0=qT[:, mt, sl],
                                    in1=CT[:, sl], op=OP.mult)
            t2 = ep.tile([128, 512], F32, tag="t2")
            nc.vector.tensor_tensor(out=t2[:], in0=psB[:], in1=STt[:, sl],
                                    op=OP.mult)
            nc.vector.tensor_tensor(out=t1[:], in0=t1[:], in1=t2[:], op=OP.add)
            nc.vector.tensor_tensor(out=qT[:, mt, sl], in0=t1[:], in1=prs[:],
                                    op=OP.mult)

    # partition-swapped copy of k rows (PE equal-base-partition rule)
    kTb = big.tile([128, 2, S], F32R, tag="kTb")
    for m in range(2):
        nc.sync.dma_start(kTb[0:64, m, :], qT[64:128, 8 + m, :])
        nc.sync.dma_start(kTb[64:128, m, :], qT[0:64, 8 + m, :])

    # ---------------- Wp prep (overlaps attention lead-in) ------------
    wpT = wbig.tile([128, 8, DIM], BF16, tag="w8", name="wpT")
    ap_bc = prep_w(h["Wp"], 8, wpT, need_alpha=True)

    stage_ctx.close()
    ps12_ctx.close()

    # late pool reuses the stage pool's SBUF space
    late = ctx.enter_context(tc.tile_pool(name="late", bufs=1))
    ybig = late.tile([128, 8, S], F32, tag="ybig")
    Dall = small.tile([16, S], F32, tag="dall")

    # ---------------- attention ----------------
    with tc.tile_pool(name="psat", bufs=1, space="PSUM") as psat:
        for hh in range(NH):
            ypo = 64 * (hh % 2)
            hk = hh // 4
            qtile, qpo = qT[:, hh // 2, :], 64 * (hh % 2)
            if qpo == 64 * (hk % 2):
                ktile, kpo = qT[:, 8 + hk // 2, :], qpo
            else:
                ktile, kpo = kTb[:, hk // 2, :], qpo

            pyh = psat.tile([HD + 1, S], F32, tag="pyh", name="pyh_t", bufs=2)
            for i in range(8):
                ex = expool.tile([128, S], F32R, tag="expT")
                psc = psat.tile([128, S], F32, tag="psc", name="psc_t",
                                bufs=2)
                ilo = i * 128
                for half in range(2):
                    lo = max(half * 512, ilo)
                    hi = half * 512 + 512
                    if lo >= hi:
                        continue
                    diag = (ilo == lo) and (i >= half * 4) and (i < half * 4 + 4)
                    nc.tensor.matmul(psc[:, lo:hi],
                                     ktile[kpo:kpo + 64, ilo:ilo + 128],
                                     qtile[qpo:qpo + 64, lo:hi],
                                     start=True, stop=not diag)
                    if diag:
                        nc.tensor.matmul(psc[:, lo:lo + 128],
                                         MASKA[:], IDB[:], start=False,
                                         stop=True)
                # one exp covering both halves [ilo:1024]
                nc.scalar.activation(out=ex[:, ilo:S], in_=psc[:, ilo:S],
                                     func=AF.Exp, scale=SCALE)
                for half in range(2):
                    lo = max(half * 512, ilo)
                    hi = half * 512 + 512
                    if lo >= hi:
                        continue
                    nc.tensor.matmul(pyh[:, lo:hi],
                                     vbuf[:, i, hk, :], ex[:, lo:hi],
                                     start=(i == 0),
                                     stop=(i == 7 or (half == 0 and i == 3)))
            # evict y rows + denominator row (DMA handles the
            # arbitrary-partition placement into Dall)
            nc.vector.tensor_copy(ybig[ypo:ypo + HD, hh // 2, :],
                                  pyh[0:HD, :])
            dtmp = small.tile([1, S], F32, tag="dtmp", bufs=1, name="dtmp")
            nc.vector.tensor_copy(dtmp[:], pyh[HD:HD + 1, :])
            nc.sync.dma_start(Dall[hh:hh + 1, :], dtmp[:])

    # ---------- post-attention: division, fake-quant, out proj --------
    with tc.tile_pool(name="psout", bufs=1, space="PSUM") as pso:
        Drect = small.tile([16, S], F32R, tag="drec")
        nc.vector.reciprocal(out=Drect[:], in_=Dall[:])
        Drec = Drect[:]
        for i in range(8):
            dx = pso.tile([128, S], F32, tag="dx", name="dx_t", bufs=2)
            for half in range(2):
                sl = slice(half * 512, half * 512 + 512)
                nc.tensor.matmul(dx[:, sl], SEL20[0:16, i, :], Drec[:, sl],
                                 start=True, stop=True)
            nc.vector.tensor_tensor(out=ybig[:, i, :], in0=ybig[:, i, :],
                                    in1=dx[:], op=OP.mult)
        # m1[p, t] = max_d |ybig[p, d, t]|
        m1 = small.tile([128, S], F32, tag="m1")
        nc.scalar.activation(out=m1[:], in_=ybig[:, 0, :], func=AF.Abs)
        for i in range(1, 8):
            ab = late.tile([128, S], F32, tag="z", bufs=2, name="ab_t")
            nc.scalar.activation(out=ab[:], in_=ybig[:, i, :], func=AF.Abs)
            nc.vector.tensor_tensor(out=m1[:], in0=m1[:], in1=ab[:],
                                    op=OP.max)
        gmy8 = small.tile([128, 8], F32, tag="gmy8")
        gyap = small.tile([128, 8], F32)
        for c in range(8):
            ptp = pso.tile([128, 128], F32, tag="po", name="ptp_t", bufs=2)
            nc.tensor.transpose(ptp[:], m1[:, c * 128:(c + 1) * 128], IDF[:])
            my = small.tile([128, 1], F32, tag="my", bufs=2)
            nc.vector.tensor_reduce(out=my[:], in_=ptp[:], axis=AX.X,
                                    op=OP.max)
            nc.vector.tensor_scalar(out=gmy8[:, c:c + 1], in0=my[:],
                                    scalar1=EPS8, scalar2=INV127,
                                    op0=OP.max, op1=OP.mult)
            nc.vector.tensor_tensor(out=gyap[:, c:c + 1],
                                    in0=gmy8[:, c:c + 1],
                                    in1=ap_bc[:], op=OP.mult)
        giv8 = small.tile([128, 8], F32R, tag="giv8")
        nc.vector.reciprocal(out=giv8[:], in_=gmy8[:])
        givr = small.tile([1, S], F32R, tag="givr")
        for c in range(8):
            pgi = pso.tile([1, 128], F32R, tag="po", name="pgi_t", bufs=2)
            nc.tensor.transpose(pgi[:], giv8[:, c:c + 1], IDR)
            nc.vector.tensor_copy(givr[:, c * 128:(c + 1) * 128], pgi[:])
        pg = pso.tile([128, S], F32, tag="pg", name="pg_t", bufs=1)
        for half in range(2):
            sl = slice(half * 512, half * 512 + 512)
            nc.tensor.matmul(pg[:, sl], ONES8[0:1, :], givr[:, sl],
                             start=True, stop=True)
        yint = i8.tile([128, 8, S], BF16, tag="i8")  # reuses xT slot
        for i in range(8):
            z = late.tile([128, S], F32, tag="z", bufs=2)
            nc.vector.tensor_tensor(out=z[:], in0=ybig[:, i, :], in1=pg[:],
                                    op=OP.mult)
            nc.vector.tensor_scalar(out=yint[:, i, :], in0=z[:],
                                    scalar1=MAGIC, scalar2=MAGIC,
                                    op0=OP.add, op1=OP.subtract)
        # ---------------- output projection ----------------
        for st in range(8):
            for half in range(2):
                sl = slice(half * 512, half * 512 + 512)
                po = pso.tile([128, 512], F32, tag="po", name="po_t", bufs=2)
                for d in range(8):
                    nc.tensor.matmul(po[:],
                                     yint[:, d, st * 128:(st + 1) * 128],
                                     wpT[:, d, sl], start=(d == 0),
                                     stop=(d == 7))
                ot = opool.tile([128, 512], F32, tag="ot")
                nc.vector.tensor_scalar(out=ot[:], in0=po[:],
                                        scalar1=gyap[:, st:st + 1],
                                        scalar2=None, op0=OP.mult)
                nc.sync.dma_start(h["out"].ap()[st * 128:(st + 1) * 128, sl],
                                  ot[:])
    ctx.close()


def kernel(**inputs):
    if "nc" not in _CACHE:
        _CACHE["nc"] = build()
    nc = _CACHE["nc"]
    x = np.ascontiguousarray(inputs["x"], dtype=np.float32)
    shared = {k: np.ascontiguousarray(inputs[k], dtype=np.float32)
              for k in ("Wq", "Wk", "Wv", "Wp")}
    qgain = np.ascontiguousarray(inputs["q_gain"], dtype=np.float32)
    in_maps = [dict(x=np.ascontiguousarray(x[b]), q_gain=qgain, **shared)
               for b in range(N_CORES)]
    res = run_bass_kernel_spmd(nc, in_maps, core_ids=list(range(N_CORES)))
    out = np.stack([res.results[b]["out"] for b in range(N_CORES)], axis=0)
    return out.astype(np.float32)
